# revision 3
# baseline (speedup 1.0000x reference)
"""Trainium2 Bass kernel for MHA with query-axis softmax (nn_MHA_2568390443327).

Reference computation (B=4, N=2048, DIM=1024, 16 heads x 64):
    qkv = x @ w_qkv ; q,k,v = split(qkv)
    scores = (q @ k^T) * scale            # [b,h,i(query),j(key)]
    attn = softmax(scores, axis=QUERY)    # normalized over i, per key j
    y = attn @ v ; out = y @ w_out + b_out

Sharding (8 cores): batch (4) x head-half (2). Each core gets its batch's
x (pre-transposed), the qkv weight columns and w_out rows for its 8 heads,
and produces a partial [DIM, N] fp16 output (transposed). Host sums the two
head-half partials per batch, adds the bias, and transposes back.

v2 design (ScalarE-saturation): the exp of all N^2 scores on the Scalar
engine (1 elem/lane/cycle) is the hard floor (~360us/core), so the whole
schedule is built to keep ACT 100% busy:
  - PSUM split: banks 0-3 = two [128,1024] score slots (exp ping-pong),
    banks 4-5 = two [128,512] y-block accumulators, banks 6-7 = two
    [128,512] projection-chain outputs. Every consumer has its own banks
    so no engine ever waits on another's buffer.
  - Scores are computed transposed S_T[j,i]; the query-axis softmax is a
    free-axis exp+accumulate (denominator) on ACT; 1/den folds into a
    per-key rescale of v (vp), never touching the NxN matrix.
  - y accumulates over j in PSUM in 4-j-tile blocks, then DVE adds the
    block into an SBUF fp16 accumulator -> y PSUM rent is 2 banks.
  - qkv/out projection matmul chains thread into the PE slack during the
    ACT-bound attention phase via the dedicated projection banks.
"""

import os
import numpy as np

# ---------------------------------------------------------------------------
# Problem constants (hardcoded; kernel.py must be self-contained).
B = 4
N = 2048          # sequence length
F = 1024          # model dim (contraction for qkv proj)
HEADS_TOT = 16
DH = 64           # head dim
HH = 8            # heads per core (head-half)
CH = HH * DH      # 512: per-core hidden
OUT = 1024        # output dim
SCALE = 0.125     # 1/sqrt(64)
N_CORES = 8

P = 128           # partitions
NC512 = 512       # matmul free-dim chunk (one PSUM bank of fp32)
S_W = 1024        # score tile width (half the i range), 2 banks
JB = 4            # j-tiles per y accumulation block


def _build_nc():
    import concourse.bass as bass  # noqa: F401
    import concourse.mybir as mybir
    from concourse import bacc
    from concourse.tile import TileContext

    f32 = mybir.dt.float32
    f16 = mybir.dt.float16
    EXP = mybir.ActivationFunctionType.Exp

    nc = bacc.Bacc(None, target_bir_lowering=False)

    xT = nc.declare_dram_parameter("xT", [F, N], f16, isOutput=False)
    wqkv = nc.declare_dram_parameter("wqkv", [F, 3 * CH], f16, isOutput=False)
    wout = nc.declare_dram_parameter("wout", [CH, OUT], f16, isOutput=False)
    outT = nc.declare_dram_parameter("outT", [OUT, N], f16, isOutput=True)

    KT = F // P            # 8 k-tiles for qkv projection contraction
    NT = N // P            # 16 j-tiles (key blocks)
    PAIRS = 4              # head pairs per core (2 heads each)
    OT = OUT // P          # 8 output row tiles
    NBLK = NT // JB        # 4 y accumulation blocks per pair

    with TileContext(nc) as tc:
        with (
            tc.tile_pool(name="p_x", bufs=1) as p_x,
            tc.tile_pool(name="p_w", bufs=1) as p_w,
            tc.tile_pool(name="p_wout", bufs=1) as p_wout,
            tc.tile_pool(name="p_v", bufs=1) as p_v,
            tc.tile_pool(name="p_qkT", bufs=1) as p_qkT,
            tc.tile_pool(name="p_ysb", bufs=1) as p_ysb,
            tc.tile_pool(name="p_at", bufs=24) as p_at,
            tc.tile_pool(name="p_den", bufs=48) as p_den,
            tc.tile_pool(name="p_vp", bufs=12) as p_vp,
            tc.tile_pool(name="p_osb", bufs=4) as p_osb,
            tc.tile_pool(name="psS", bufs=2, space="PSUM") as psS,
            tc.tile_pool(name="psY", bufs=2, space="PSUM") as psY,
            tc.tile_pool(name="psP", bufs=2, space="PSUM") as psP,
        ):
            # ---------------- persistent SBUF tiles ----------------
            xt = [p_x.tile([P, N], f16, tag=f"x{k}", name=f"x{k}")
                  for k in range(KT)]
            wt = [p_w.tile([P, 3 * CH], f16, tag=f"w{k}", name=f"w{k}")
                  for k in range(KT)]
            wout_sb = [p_wout.tile([P, OUT], f16, tag=f"wo{c}", name=f"wo{c}")
                       for c in range(PAIRS)]
            vnat = [p_v.tile([P, CH], f16, tag=f"v{j}", name=f"v{j}")
                    for j in range(NT)]
            # q/k for current+next pair (double-buffered across pairs)
            qT = [p_qkT.tile([P, N], f16, tag=f"qT{i}", name=f"qT{i}")
                  for i in range(2)]
            kT = [p_qkT.tile([P, N], f16, tag=f"kT{i}", name=f"kT{i}")
                  for i in range(2)]
            # per-pair fp16 y accumulators (all pairs kept for out-proj)
            y_sb = [p_ysb.tile([P, N], f16, tag=f"y{p_}", name=f"y{p_}")
                    for p_ in range(PAIRS)]

            for k in range(KT):
                nc.sync.dma_start(out=wt[k], in_=wqkv[k * P:(k + 1) * P, :])
                nc.sync.dma_start(out=xt[k], in_=xT[k * P:(k + 1) * P, :])
            for c in range(PAIRS):
                nc.sync.dma_start(out=wout_sb[c],
                                  in_=wout[c * P:(c + 1) * P, :])

            # ---------------- projection helpers ----------------
            def emit_v_proj(j):
                """vnat[j][n, c] = sum_f xT[f, n] w_v[f, c], f16 out."""
                ps = psP.tile([P, NC512], f32, tag="pp", name=f"pv{j}")
                for k in range(KT):
                    nc.tensor.matmul(
                        ps,
                        lhsT=xt[k][:, j * P:(j + 1) * P],
                        rhs=wt[k][:, 2 * CH:3 * CH],
                        start=(k == 0), stop=(k == KT - 1))
                nc.vector.tensor_copy(vnat[j], ps)

            def emit_qk_chain(pr, sec, ch):
                """One 512-wide chain of the q(sec=0)/k(sec=1) projection for
                pair pr: dst[128 rows, 512 i] accumulated over 8 k-tiles."""
                dst = (qT if sec == 0 else kT)[pr % 2]
                ps = psP.tile([P, NC512], f32, tag="pp",
                              name=f"qk{pr}_{sec}_{ch}")
                i0 = ch * NC512
                for k in range(KT):
                    nc.tensor.matmul(
                        ps,
                        lhsT=wt[k][:, sec * CH + pr * P:sec * CH + (pr + 1) * P],
                        rhs=xt[k][:, i0:i0 + NC512],
                        start=(k == 0), stop=(k == KT - 1))
                nc.vector.tensor_copy(dst[:, i0:i0 + NC512], ps)

            def emit_out_chain(o, ich):
                """outT[o-block, i-chunk] = sum_c wout[c,o] y[c,i] over all
                4 pairs' y_sb tiles."""
                ps = psP.tile([P, NC512], f32, tag="pp", name=f"po{o}_{ich}")
                for c in range(PAIRS):
                    nc.tensor.matmul(
                        ps,
                        lhsT=wout_sb[c][:, o * P:(o + 1) * P],
                        rhs=y_sb[c][:, ich * NC512:(ich + 1) * NC512],
                        start=(c == 0), stop=(c == PAIRS - 1))
                osb = p_osb.tile([P, NC512], f16, tag="osb", name="osb")
                nc.vector.tensor_copy(osb, ps)
                nc.sync.dma_start(
                    out=outT[o * P:(o + 1) * P,
                             ich * NC512:(ich + 1) * NC512],
                    in_=osb)

            # ---------------- prelude ----------------
            # q/k projection of pair 0 and the first y-block's v tiles; the
            # Tile scheduler starts each chain as its DMA deps land.
            for ch in range(4):
                emit_qk_chain(0, 1, ch)      # k first (scores lhsT)
            for ch in range(4):
                emit_qk_chain(0, 0, ch)
            for j in range(JB):
                emit_v_proj(j)

            # Filler generator: projection work to thread into PE slack
            # during the ACT-bound attention phase of each pair.
            def filler_gen(pr):
                # during pair pr: v-proj for upcoming blocks (pair 0 only),
                # q/k proj of pair pr+1.
                work = []
                if pr == 0:
                    for j in range(JB, NT):
                        work.append(("v", j))
                if pr + 1 < PAIRS:
                    for ch in range(4):
                        work.append(("k", ch))
                    for ch in range(4):
                        work.append(("q", ch))
                return work

            # ---------------- attention ----------------
            for pr in range(PAIRS):
                filler = filler_gen(pr)
                fill_per_j = (len(filler) + NT - 1) // NT
                fi = 0
                ats = {}    # (ho, j, half) -> at tile
                vp = {}     # (ho, j) -> vp tile
                for j in range(NT):
                    js = slice(j * P, (j + 1) * P)
                    dens = {}
                    # 4 exps per j: (A,i0) (A,i1) (B,i0) (B,i1); psS bufs=2
                    # gives the S0/S1 ping-pong that keeps ACT back-to-back.
                    for ho in (0, DH):
                        for hf in range(2):
                            s = psS.tile([P, S_W], f32, tag="ss",
                                         name=f"s{pr}_{j}_{ho}_{hf}")
                            for c2 in range(2):
                                i0 = hf * S_W + c2 * NC512
                                nc.tensor.matmul(
                                    s[:, c2 * NC512:(c2 + 1) * NC512],
                                    lhsT=kT[pr % 2][ho:ho + DH, js],
                                    rhs=qT[pr % 2][ho:ho + DH, i0:i0 + NC512],
                                    start=True, stop=True,
                                    tile_position=(ho, 0))
                            at = p_at.tile([P, S_W], f16, tag="at",
                                           name=f"at{ho}_{hf}")
                            den = p_den.tile([P, 1], f32, tag="den",
                                             name=f"den{ho}_{hf}")
                            nc.scalar.activation(at, s, EXP,
                                                 scale=SCALE, accum_out=den)
                            ats[(ho, j, hf)] = at
                            dens[(ho, hf)] = den
                    # denominator -> reciprocal -> rescaled v rows
                    for ho in (0, DH):
                        dtot = p_den.tile([P, 1], f32, tag="den", name="dtot")
                        nc.vector.tensor_add(dtot, dens[(ho, 0)],
                                             dens[(ho, 1)])
                        rec = p_den.tile([P, 1], f32, tag="den", name="rec")
                        nc.vector.reciprocal(rec, dtot)
                        vpt = p_vp.tile([P, DH], f16, tag="vp",
                                        name=f"vp{ho}")
                        c0 = pr * 2 * DH + ho
                        nc.vector.tensor_scalar_mul(
                            vpt, vnat[j][:, c0:c0 + DH], rec)
                        vp[(ho, j)] = vpt
                    # thread projection filler into this j-step's PE slack
                    for _ in range(fill_per_j):
                        if fi < len(filler):
                            kind, a = filler[fi]
                            fi += 1
                            if kind == "v":
                                emit_v_proj(a)
                            elif kind == "k":
                                emit_qk_chain(pr + 1, 1, a)
                            else:
                                emit_qk_chain(pr + 1, 0, a)
                    # y block: every JB j-tiles, accumulate into PSUM then
                    # DVE-add into the pair's fp16 SBUF accumulator.
                    if j % JB == JB - 1:
                        blk = j // JB
                        jlist = list(range(blk * JB, (blk + 1) * JB))
                        for ic in range(4):
                            hf, c2 = divmod(ic, 2)
                            yb = psY.tile([P, NC512], f32, tag="yy",
                                          name=f"yb{pr}_{blk}_{ic}")
                            for idx, jj in enumerate(jlist):
                                for ho in (0, DH):
                                    nc.tensor.matmul(
                                        yb[ho:ho + DH, :],
                                        lhsT=vp[(ho, jj)],
                                        rhs=ats[(ho, jj, hf)][:, c2 * NC512:
                                                              (c2 + 1) * NC512],
                                        start=(idx == 0),
                                        stop=(idx == JB - 1),
                                        tile_position=(0, ho))
                            i0 = hf * S_W + c2 * NC512
                            if blk == 0:
                                nc.vector.tensor_copy(
                                    y_sb[pr][:, i0:i0 + NC512], yb)
                            else:
                                nc.vector.tensor_add(
                                    y_sb[pr][:, i0:i0 + NC512],
                                    y_sb[pr][:, i0:i0 + NC512], yb)
                        # release consumed at/vp tiles
                        for jj in jlist:
                            for ho in (0, DH):
                                ats.pop((ho, jj, 0), None)
                                ats.pop((ho, jj, 1), None)
                                vp.pop((ho, jj), None)

            # ---------------- output projection (tail) ----------------
            for o in range(OT):
                for ich in range(N // NC512):
                    emit_out_chain(o, ich)
    return nc


def _shard_inputs(x, w_qkv, w_out):
    """Build per-core input maps: core c -> (batch c//2, head-half c%2)."""
    in_maps = []
    for c in range(N_CORES):
        b, hh = c // 2, c % 2
        cols = slice(hh * CH, (hh + 1) * CH)
        xTc = np.ascontiguousarray(np.asarray(x[b]).T, dtype=np.float16)
        wq = w_qkv[:, 0 * F:1 * F][:, cols]
        wk = w_qkv[:, 1 * F:2 * F][:, cols]
        wv = w_qkv[:, 2 * F:3 * F][:, cols]
        wqkv_c = np.ascontiguousarray(
            np.concatenate([wq, wk, wv], axis=1), dtype=np.float16)
        wout_c = np.ascontiguousarray(w_out[cols, :], dtype=np.float16)
        in_maps.append({"xT": xTc, "wqkv": wqkv_c, "wout": wout_c})
    return in_maps


def _gather_outputs(results, b_out):
    out = np.empty((B, N, OUT), np.float32)
    bias = np.asarray(b_out, dtype=np.float32)
    for b in range(B):
        acc = (results[2 * b]["outT"].astype(np.float32)
               + results[2 * b + 1]["outT"].astype(np.float32))  # [OUT, N]
        out[b] = acc.T + bias
    return out


# Test instrumentation (harness just calls kernel(); these stay default).
_TRACE = False
_LAST_RESULT = None


def kernel(x, w_qkv, w_out, b_out):
    global _LAST_RESULT
    # The bass->PJRT path needs the axon trn2 devices visible to jax.
    if os.environ.get("JAX_PLATFORMS") not in (None, "", "axon"):
        os.environ.pop("JAX_PLATFORMS", None)
    from concourse.bass_utils import run_bass_kernel_spmd

    nc = _build_nc()
    if not nc.is_finalized():
        nc.finalize()  # runs Bacc legalization (wait splitting, reg alloc)
    in_maps = _shard_inputs(np.asarray(x), np.asarray(w_qkv),
                            np.asarray(w_out))
    res = run_bass_kernel_spmd(nc, in_maps, list(range(N_CORES)),
                               trace=_TRACE)
    _LAST_RESULT = res
    return _gather_outputs(res.results, np.asarray(b_out))


# ---------------------------------------------------------------------------
# Numpy emulation of the per-core device program (for host-logic testing;
# not used by kernel()).
def _emulate_core(m):
    xT, wqkv, wout = m["xT"], m["wqkv"], m["wout"]
    qT = (wqkv[:, 0:CH].T @ xT)          # [CH, N]
    kTm = (wqkv[:, CH:2 * CH].T @ xT)    # [CH, N]
    v = xT.T @ wqkv[:, 2 * CH:3 * CH]    # [N, CH]
    y = np.empty((CH, N), np.float32)
    for h in range(HH):
        qh = qT[h * DH:(h + 1) * DH, :]      # [DH, N(i)]
        kh = kTm[h * DH:(h + 1) * DH, :]     # [DH, N(j)]
        sT = kh.T @ qh                       # [j, i]
        e = np.exp(sT * SCALE)
        den = e.sum(axis=1, keepdims=True)   # over queries i, per key j
        vpm = v[:, h * DH:(h + 1) * DH] / den
        y[h * DH:(h + 1) * DH, :] = vpm.T @ e  # [DH, i]
    outT_acc = wout.T @ y                    # [OUT, N]
    return outT_acc.astype(np.float16)


def _kernel_emulated(x, w_qkv, w_out, b_out):
    in_maps = _shard_inputs(np.asarray(x), np.asarray(w_qkv),
                            np.asarray(w_out))
    results = [{"outT": _emulate_core(m)} for m in in_maps]
    return _gather_outputs(results, np.asarray(b_out))


# revision 7
# speedup vs baseline: 1.0819x; 1.0819x over previous
"""Trainium2 Bass kernel for MHA with query-axis softmax (nn_MHA_2568390443327).

Reference computation (B=4, N=2048, DIM=1024, 16 heads x 64):
    qkv = x @ w_qkv ; q,k,v = split(qkv)
    scores = (q @ k^T) * scale            # [b,h,i(query),j(key)]
    attn = softmax(scores, axis=QUERY)    # normalized over i, per key j
    y = attn @ v ; out = y @ w_out + b_out

Sharding (8 cores): batch (4) x head-half (2). Each core gets its batch's
x (pre-transposed), the qkv weight columns and w_out rows for its 8 heads,
and produces a partial [DIM, N] fp16 output (transposed). Host sums the two
head-half partials per batch, adds the bias, and transposes back.

v3 design (ScalarE-saturation): the exp of all N^2 scores on the Scalar
engine (1 elem/lane/cycle) is the hard floor (~360us/core), so the whole
schedule keeps ACT busy:
  - PSUM split: banks 0-3 = two [128,1024] score slots (exp ping-pong),
    banks 4-5 = two [128,512] y-block accumulators, banks 6-7 = two
    projection-chain outputs. Every consumer has its own banks.
  - Scores are computed transposed S_T[j,i]; the query-axis softmax is a
    free-axis exp+accumulate (denominator) on ACT; 1/den folds into a
    per-key rescale of v (vp), never touching the NxN matrix.
  - y accumulates over j in PSUM in 4-j-tile blocks; each block is emitted
    one block LATE, one i-chunk per j-step, right after that step's score
    fills, so y matmuls never sit ahead of ready score fills in the PE's
    FIFO queue (which would stall ACT).
  - qkv/out projection chains thread into PE slack at 256-free granularity
    (short enough to never delay a score fill by more than ~1us).
"""

import os
import numpy as np

# ---------------------------------------------------------------------------
# Problem constants (hardcoded; kernel.py must be self-contained).
B = 4
N = 2048          # sequence length
F = 1024          # model dim (contraction for qkv proj)
HEADS_TOT = 16
DH = 64           # head dim
HH = 8            # heads per core (head-half)
CH = HH * DH      # 512: per-core hidden
OUT = 1024        # output dim
SCALE = 0.125     # 1/sqrt(64)
N_CORES = 8

P = 128           # partitions
NC512 = 512       # matmul free-dim chunk (one PSUM bank of fp32)
S_W = 1024        # score tile width (half the i range), 2 banks
JB = 4            # j-tiles per y accumulation block


def _build_nc():
    import concourse.bass as bass  # noqa: F401
    import concourse.mybir as mybir
    from concourse import bacc
    from concourse.tile import TileContext

    f32 = mybir.dt.float32
    f16 = mybir.dt.float16
    EXP = mybir.ActivationFunctionType.Exp

    nc = bacc.Bacc(None, target_bir_lowering=False)

    xT = nc.declare_dram_parameter("xT", [F, N], f16, isOutput=False)
    wqkv = nc.declare_dram_parameter("wqkv", [F, 3 * CH], f16, isOutput=False)
    wout = nc.declare_dram_parameter("wout", [CH, OUT], f16, isOutput=False)
    outT = nc.declare_dram_parameter("outT", [OUT, N], f16, isOutput=True)

    KT = F // P            # 8 k-tiles for qkv projection contraction
    NT = N // P            # 16 j-tiles (key blocks)
    PAIRS = 4              # head pairs per core (2 heads each)
    OT = OUT // P          # 8 output row tiles

    with TileContext(nc) as tc:
        with (
            tc.tile_pool(name="p_x", bufs=1) as p_x,
            tc.tile_pool(name="p_w", bufs=1) as p_w,
            tc.tile_pool(name="p_wout", bufs=1) as p_wout,
            tc.tile_pool(name="p_v", bufs=1) as p_v,
            tc.tile_pool(name="p_qkT", bufs=1) as p_qkT,
            tc.tile_pool(name="p_ysb", bufs=1) as p_ysb,
            tc.tile_pool(name="p_at", bufs=34) as p_at,
            tc.tile_pool(name="p_den", bufs=64) as p_den,
            tc.tile_pool(name="p_vp", bufs=20) as p_vp,
            tc.tile_pool(name="p_osb", bufs=4) as p_osb,
            tc.tile_pool(name="psS", bufs=2, space="PSUM") as psS,
            tc.tile_pool(name="psY", bufs=2, space="PSUM") as psY,
            tc.tile_pool(name="psP", bufs=2, space="PSUM") as psP,
        ):
            # ---------------- persistent SBUF tiles ----------------
            xt = [p_x.tile([P, N], f16, tag=f"x{k}", name=f"x{k}")
                  for k in range(KT)]
            wt = [p_w.tile([P, 3 * CH], f16, tag=f"w{k}", name=f"w{k}")
                  for k in range(KT)]
            wout_sb = [p_wout.tile([P, OUT], f16, tag=f"wo{c}", name=f"wo{c}")
                       for c in range(PAIRS)]
            vnat = [p_v.tile([P, CH], f16, tag=f"v{j}", name=f"v{j}")
                    for j in range(NT)]
            # q/k for current+next pair (double-buffered across pairs)
            qT = [p_qkT.tile([P, N], f16, tag=f"qT{i}", name=f"qT{i}")
                  for i in range(2)]
            kT = [p_qkT.tile([P, N], f16, tag=f"kT{i}", name=f"kT{i}")
                  for i in range(2)]
            # per-pair fp16 y accumulators (all pairs kept for out-proj)
            y_sb = [p_ysb.tile([P, N], f16, tag=f"y{p_}", name=f"y{p_}")
                    for p_ in range(PAIRS)]

            for k in range(KT):
                nc.sync.dma_start(out=wt[k], in_=wqkv[k * P:(k + 1) * P, :])
                nc.sync.dma_start(out=xt[k], in_=xT[k * P:(k + 1) * P, :])
            for c in range(PAIRS):
                nc.sync.dma_start(out=wout_sb[c],
                                  in_=wout[c * P:(c + 1) * P, :])

            # ---------------- projection helpers ----------------
            def emit_v_proj(j, half, width):
                """vnat[j][n, c-chunk] = sum_f xT[f, n] w_v[f, c-chunk]."""
                ps = psP.tile([P, width], f32, tag="pp",
                              name=f"pv{j}_{half}")
                c0 = half * width
                for k in range(KT):
                    nc.tensor.matmul(
                        ps,
                        lhsT=xt[k][:, j * P:(j + 1) * P],
                        rhs=wt[k][:, 2 * CH + c0:2 * CH + c0 + width],
                        start=(k == 0), stop=(k == KT - 1))
                nc.vector.tensor_copy(vnat[j][:, c0:c0 + width], ps)

            def emit_qk_chain(pr, sec, ch, width):
                """One `width`-wide chain of the q(sec=0)/k(sec=1) projection
                for pair pr: dst[128 rows, width i] over 8 k-tiles."""
                dst = (qT if sec == 0 else kT)[pr % 2]
                ps = psP.tile([P, width], f32, tag="pp",
                              name=f"qk{pr}_{sec}_{ch}")
                i0 = ch * width
                for k in range(KT):
                    nc.tensor.matmul(
                        ps,
                        lhsT=wt[k][:, sec * CH + pr * P:sec * CH + (pr + 1) * P],
                        rhs=xt[k][:, i0:i0 + width],
                        start=(k == 0), stop=(k == KT - 1))
                nc.vector.tensor_copy(dst[:, i0:i0 + width], ps)

            def emit_out_chain(o, ich):
                """outT[o-block, i-chunk] = sum_c wout[c,o] y[c,i] over all
                4 pairs' y_sb tiles."""
                ps = psP.tile([P, NC512], f32, tag="pp", name=f"po{o}_{ich}")
                for c in range(PAIRS):
                    nc.tensor.matmul(
                        ps,
                        lhsT=wout_sb[c][:, o * P:(o + 1) * P],
                        rhs=y_sb[c][:, ich * NC512:(ich + 1) * NC512],
                        start=(c == 0), stop=(c == PAIRS - 1))
                osb = p_osb.tile([P, NC512], f16, tag="osb", name="osb")
                nc.vector.tensor_copy(osb, ps)
                nc.sync.dma_start(
                    out=outT[o * P:(o + 1) * P,
                             ich * NC512:(ich + 1) * NC512],
                    in_=osb)

            # ---------------- prelude ----------------
            # Minimal chains before the first exp can start: k keys 0..511
            # (j-tiles 0-3) and the full q i-range for pair 0.
            emit_qk_chain(0, 1, 0, 512)
            for ch in range(4):
                emit_qk_chain(0, 0, ch, 512)
            for ch in range(1, 4):
                emit_qk_chain(0, 1, ch, 512)
            for j in range(JB):
                emit_v_proj(j, 0, 512)

            # Filler: projection work threaded into PE slack during the
            # ACT-bound attention phase, at 256-free granularity.
            def filler_list(pr):
                work = []
                if pr == 0:
                    for j in range(JB, NT):
                        for h in range(2):
                            work.append(("v", j, h))
                if pr + 1 < PAIRS:
                    for ch in range(8):
                        work.append(("k", pr + 1, ch))
                    for ch in range(8):
                        work.append(("q", pr + 1, ch))
                return work

            # ---------------- attention ----------------
            ats = {}    # (pr, ho, j, half) -> at tile
            vp = {}     # (pr, ho, j) -> vp tile
            y_queue = []  # pending y-chunk closures, emitted 1 per j-step

            def make_y_chunk(pr, blk, ic):
                jlist = list(range(blk * JB, (blk + 1) * JB))

                def emit():
                    hf, c2 = divmod(ic, 2)
                    yb = psY.tile([P, NC512], f32, tag="yy",
                                  name=f"yb{pr}_{blk}_{ic}")
                    for idx, jj in enumerate(jlist):
                        for ho in (0, DH):
                            nc.tensor.matmul(
                                yb[ho:ho + DH, :],
                                lhsT=vp[(pr, ho, jj)],
                                rhs=ats[(pr, ho, jj, hf)][:, c2 * NC512:
                                                          (c2 + 1) * NC512],
                                start=(idx == 0),
                                stop=(idx == JB - 1),
                                tile_position=(0, ho))
                    i0 = hf * S_W + c2 * NC512
                    if blk == 0:
                        nc.vector.tensor_copy(y_sb[pr][:, i0:i0 + NC512], yb)
                    else:
                        nc.vector.tensor_add(
                            y_sb[pr][:, i0:i0 + NC512],
                            y_sb[pr][:, i0:i0 + NC512], yb)
                    if ic == 3:
                        for jj in jlist:
                            for ho in (0, DH):
                                ats.pop((pr, ho, jj, 0), None)
                                ats.pop((pr, ho, jj, 1), None)
                                vp.pop((pr, ho, jj), None)
                return emit

            def emit_filler(item):
                kind, a, b_ = item
                if kind == "v":
                    emit_v_proj(a, b_, 256)
                elif kind == "k":
                    emit_qk_chain(a, 1, b_, 256)
                else:
                    emit_qk_chain(a, 0, b_, 256)

            for pr in range(PAIRS):
                filler = filler_list(pr)
                per_j = (len(filler) + NT - 1) // NT
                fi = 0
                for j in range(NT):
                    # thread one filler chain ahead of this step's fills
                    if fi < len(filler):
                        emit_filler(filler[fi])
                        fi += 1
                    js = slice(j * P, (j + 1) * P)
                    dens = {}
                    # 4 exps per j: (A,i0) (A,i1) (B,i0) (B,i1); psS bufs=2
                    # gives the S0/S1 ping-pong that keeps ACT back-to-back.
                    for ho in (0, DH):
                        for hf in range(2):
                            s = psS.tile([P, S_W], f32, tag="ss",
                                         name=f"s{pr}_{j}_{ho}_{hf}")
                            for c2 in range(2):
                                i0 = hf * S_W + c2 * NC512
                                nc.tensor.matmul(
                                    s[:, c2 * NC512:(c2 + 1) * NC512],
                                    lhsT=kT[pr % 2][ho:ho + DH, js],
                                    rhs=qT[pr % 2][ho:ho + DH, i0:i0 + NC512],
                                    start=True, stop=True,
                                    tile_position=(ho, 0))
                            at = p_at.tile([P, S_W], f16, tag="at",
                                           name=f"at{ho}_{hf}")
                            den = p_den.tile([P, 1], f32, tag="den",
                                             name=f"den{ho}_{hf}")
                            nc.scalar.activation(at, s, EXP,
                                                 scale=SCALE, accum_out=den)
                            ats[(pr, ho, j, hf)] = at
                            dens[(ho, hf)] = den
                    # one pending y-chunk (from the previous block) per step
                    if y_queue:
                        y_queue.pop(0)()
                    # rest of this step's filler quota, behind the y work
                    for _ in range(per_j - 1):
                        if fi < len(filler):
                            emit_filler(filler[fi])
                            fi += 1
                    # denominator -> reciprocal -> rescaled v rows
                    for ho in (0, DH):
                        dtot = p_den.tile([P, 1], f32, tag="den", name="dtot")
                        nc.vector.tensor_add(dtot, dens[(ho, 0)],
                                             dens[(ho, 1)])
                        rec = p_den.tile([P, 1], f32, tag="den", name="rec")
                        nc.vector.reciprocal(rec, dtot)
                        vpt = p_vp.tile([P, DH], f16, tag="vp",
                                        name=f"vp{ho}")
                        c0 = pr * 2 * DH + ho
                        nc.vector.tensor_scalar_mul(
                            vpt, vnat[j][:, c0:c0 + DH], rec)
                        vp[(pr, ho, j)] = vpt
                    if j % JB == JB - 1:
                        blk = j // JB
                        for ic in range(4):
                            y_queue.append(make_y_chunk(pr, blk, ic))
                # any filler not emitted by the per-step quota
                while fi < len(filler):
                    emit_filler(filler[fi])
                    fi += 1

            # drain remaining y chunks (last block of last pair)
            while y_queue:
                y_queue.pop(0)()

            # ---------------- output projection (tail) ----------------
            for o in range(OT):
                for ich in range(N // NC512):
                    emit_out_chain(o, ich)
    return nc


def _shard_inputs(x, w_qkv, w_out):
    """Build per-core input maps: core c -> (batch c//2, head-half c%2)."""
    in_maps = []
    for c in range(N_CORES):
        b, hh = c // 2, c % 2
        cols = slice(hh * CH, (hh + 1) * CH)
        xTc = np.ascontiguousarray(np.asarray(x[b]).T, dtype=np.float16)
        wq = w_qkv[:, 0 * F:1 * F][:, cols]
        wk = w_qkv[:, 1 * F:2 * F][:, cols]
        wv = w_qkv[:, 2 * F:3 * F][:, cols]
        wqkv_c = np.ascontiguousarray(
            np.concatenate([wq, wk, wv], axis=1), dtype=np.float16)
        wout_c = np.ascontiguousarray(w_out[cols, :], dtype=np.float16)
        in_maps.append({"xT": xTc, "wqkv": wqkv_c, "wout": wout_c})
    return in_maps


def _gather_outputs(results, b_out):
    out = np.empty((B, N, OUT), np.float32)
    bias = np.asarray(b_out, dtype=np.float32)
    for b in range(B):
        acc = (results[2 * b]["outT"].astype(np.float32)
               + results[2 * b + 1]["outT"].astype(np.float32))  # [OUT, N]
        out[b] = acc.T + bias
    return out


# Test instrumentation (harness just calls kernel(); these stay default).
_TRACE = False
_LAST_RESULT = None


def kernel(x, w_qkv, w_out, b_out):
    global _LAST_RESULT
    # The bass->PJRT path needs the axon trn2 devices visible to jax.
    if os.environ.get("JAX_PLATFORMS") not in (None, "", "axon"):
        os.environ.pop("JAX_PLATFORMS", None)
    from concourse.bass_utils import run_bass_kernel_spmd

    nc = _build_nc()
    if not nc.is_finalized():
        nc.finalize()  # runs Bacc legalization (wait splitting, reg alloc)
    in_maps = _shard_inputs(np.asarray(x), np.asarray(w_qkv),
                            np.asarray(w_out))
    res = run_bass_kernel_spmd(nc, in_maps, list(range(N_CORES)),
                               trace=_TRACE)
    _LAST_RESULT = res
    return _gather_outputs(res.results, np.asarray(b_out))


# ---------------------------------------------------------------------------
# Numpy emulation of the per-core device program (for host-logic testing;
# not used by kernel()).
def _emulate_core(m):
    xT, wqkv, wout = m["xT"], m["wqkv"], m["wout"]
    qT = (wqkv[:, 0:CH].T @ xT)          # [CH, N]
    kTm = (wqkv[:, CH:2 * CH].T @ xT)    # [CH, N]
    v = xT.T @ wqkv[:, 2 * CH:3 * CH]    # [N, CH]
    y = np.empty((CH, N), np.float32)
    for h in range(HH):
        qh = qT[h * DH:(h + 1) * DH, :]      # [DH, N(i)]
        kh = kTm[h * DH:(h + 1) * DH, :]     # [DH, N(j)]
        sT = kh.T @ qh                       # [j, i]
        e = np.exp(sT * SCALE)
        den = e.sum(axis=1, keepdims=True)   # over queries i, per key j
        vpm = v[:, h * DH:(h + 1) * DH] / den
        y[h * DH:(h + 1) * DH, :] = vpm.T @ e  # [DH, i]
    outT_acc = wout.T @ y                    # [OUT, N]
    return outT_acc.astype(np.float16)


def _kernel_emulated(x, w_qkv, w_out, b_out):
    in_maps = _shard_inputs(np.asarray(x), np.asarray(w_qkv),
                            np.asarray(w_out))
    results = [{"outT": _emulate_core(m)} for m in in_maps]
    return _gather_outputs(results, np.asarray(b_out))
